# revision 1
# baseline (speedup 1.0000x reference)
"""LSTM cell (4096x1024, H=1024) as a Bass/Tile kernel on 8 TRN2 NeuronCores.

Sharding: 2D grid — 4 batch-quarters x 2 H-halves. Core c = 2*i + j gets
batch rows [i*1024,(i+1)*1024) and gate-output columns [j*512,(j+1)*512).
Each core computes gates = x @ W_j + h_prev @ U_j + b_j for its H-half
(gate order cbar, i, f, o), then c = f*c_prev + i*cbar, h = o*tanh(c).
No collectives: the host scatters inputs and gathers the output shards.

Per-core dataflow:
  - x/h row-blocks are transposed on the PE via matmul-against-identity
    (xT tiles become the stationary operand of the gate GEMMs).
  - Weights stream per gate phase ([128,512] tiles); each (gate, m) PSUM
    chain is seeded with a K=1 ones-row matmul that broadcasts the bias.
  - ACT applies sigmoid/tanh straight out of PSUM; DVE does the gating.
"""

import numpy as np
from contextlib import ExitStack

import bass_rust
import concourse.bass as bass
import concourse.mybir as mybir
import concourse.tile as tile
from concourse.vector_clock import ScopedClock
from concourse.masks import make_identity
from concourse.bass_utils import run_bass_kernel_spmd

f32 = mybir.dt.float32
AFT = mybir.ActivationFunctionType
P = 128

B, E, H = 4096, 1024, 1024
BB, BH = 4, 2              # batch quarters x H halves
BL = B // BB               # 1024 rows per core
HL = H // BH               # 512 gate cols per core
NG = 4                     # gate order: cbar, i, f, o


class PatchedTC(tile.TileContext):
    # This neuronxcc's core_v3 CTRL (Drain/NoOp) struct carries no sync-wait
    # slots, so the Tile tail-drain's waits must ride on EVSEM instructions.
    def _drain_and_barrier(self, tick_clock, wait_clock):
        tmp = mybir.InstNoOp(name=f"I-{self.nc.next_id()}",
                             engine=mybir.EngineType.SP)
        wait_clock.add_sem_waits(tmp, ScopedClock({None: tick_clock.global_clock}))
        sync = tmp.sync_info
        if sync is not None:
            for w in sync.on_wait:
                sem = bass_rust.SemaphoreHandle(w.ant_name, w.id)
                self.nc.sync.wait_ge(sem, w.wait_value)
        self.nc.sync.drain()
        self.nc.all_engine_barrier()
        popped = self.nc._tile_sem_poison_stack.pop()
        assert popped is self._sem_poison
        self.nc.clear_and_free_semaphores(list(self.sems.allocated().values()))
        self.nc.all_engine_barrier()


_SPLIT_SEQ = [0]


def split_multiwaits(nc, default_max=1, opcode_max=None):
    """This walrus build accepts at most one sync wait per instruction (zero
    for CTRL-struct ops like Drain/NoOp). Move excess waits onto dedicated
    EventSemaphore instructions inserted just before, on the same engine —
    semantically identical on an in-order engine queue."""
    opcode_max = opcode_max or {"Drain": 0, "NoOp": 0}
    for fn in nc.m.functions:
        for blk in fn.blocks:
            cur = blk.instructions
            out, changed = [], False
            for ins in cur:
                si = ins.sync_info
                waits = list(si.on_wait) if si is not None and si.on_wait else []
                cap = opcode_max.get(ins.opcode, default_max)
                if len(waits) > cap:
                    keep = waits[len(waits) - cap:] if cap else []
                    spill = waits[:len(waits) - cap]
                    for w in spill:
                        _SPLIT_SEQ[0] += 1
                        ev = mybir.InstEventSemaphore(
                            name=f"I-evw{_SPLIT_SEQ[0]}", engine=ins.engine)
                        ev.sync_info = bass_rust.SyncInfo(
                            on_wait=[w], on_update=[])
                        out.append(ev)
                    ins.sync_info = bass_rust.SyncInfo(
                        on_wait=keep, on_update=list(si.on_update))
                    changed = True
                out.append(ins)
            if changed:
                blk.instructions = out
    return nc


def build_nc(bl=BL, e=E, h=H, hl=HL, wu_bufs=3, split=True, repeat=1):
    ke, kh, m_tiles = e // P, h // P, bl // P
    nc = bass.Bass(target_bir_lowering=False)
    x_d = nc.declare_dram_parameter("x", [bl, e], f32, isOutput=False)
    h_d = nc.declare_dram_parameter("h_prev", [bl, h], f32, isOutput=False)
    c_d = nc.declare_dram_parameter("c_prev", [bl, hl], f32, isOutput=False)
    w_d = nc.declare_dram_parameter("w", [NG, e, hl], f32, isOutput=False)
    u_d = nc.declare_dram_parameter("u", [NG, h, hl], f32, isOutput=False)
    b_d = nc.declare_dram_parameter("b", [1, NG * hl], f32, isOutput=False)
    hout_d = nc.declare_dram_parameter("h_out", [bl, hl], f32, isOutput=True)
    cout_d = nc.declare_dram_parameter("c_out", [bl, hl], f32, isOutput=True)

    with PatchedTC(nc) as tc:
        with ExitStack() as ctx:
            const = ctx.enter_context(tc.tile_pool(name="const", bufs=1))
            persist = ctx.enter_context(tc.tile_pool(name="persist", bufs=1))
            stage = ctx.enter_context(tc.tile_pool(name="stage", bufs=3))
            wu = ctx.enter_context(tc.tile_pool(name="wu", bufs=wu_bufs))
            work = ctx.enter_context(tc.tile_pool(name="work", bufs=3))
            gpsum = ctx.enter_context(
                tc.tile_pool(name="gpsum", bufs=4, space="PSUM"))
            tpsum = ctx.enter_context(
                tc.tile_pool(name="tpsum", bufs=4, space="PSUM"))

            ident = const.tile([P, P], f32)
            make_identity(nc, ident[:])
            ones1 = const.tile([1, P], f32)
            nc.gpsimd.memset(ones1[:], 1.0)
            bias_row = const.tile([1, NG * hl], f32)
            nc.sync.dma_start(bias_row[:, :], b_d[:, :])

            def emit_body():
                # Persistent transposed activations and gating intermediates.
                xT = persist.tile([P, ke * m_tiles * P], f32)   # block (k,m) at col (k*m_tiles+m)*P
                hT = persist.tile([P, kh * m_tiles * P], f32)
                cbar = persist.tile([P, m_tiles * hl], f32)     # tanh(cand); then i*cbar in place
                tnc = persist.tile([P, m_tiles * hl], f32)      # tanh(c)

                # Gate-0 weights first so the first GEMM phase can start early.
                # One batched DMA per (gate, matrix): [1024,512] DRAM block lands
                # as 8 k-tiles side by side ([128, 8, 512]) — 2 MiB per dma_start
                # instead of 8x256 KB (dma_start fixed cost is ~1-2 us, HWDGE
                # ring is FIFO).
                w_sb = [None] * NG
                u_sb = [None] * NG

                def load_wu(g):
                    tw = wu.tile([P, ke, hl], f32, tag="wu")
                    nc.sync.dma_start(
                        tw[:], w_d[g].rearrange("(k p) c -> p k c", p=P))
                    w_sb[g] = tw
                    tu = wu.tile([P, kh, hl], f32, tag="wu")
                    nc.sync.dma_start(
                        tu[:], u_d[g].rearrange("(k p) c -> p k c", p=P))
                    u_sb[g] = tu

                def transpose_rows(src_d, dst, n_k, m):
                    st = stage.tile([P, n_k * P], f32, tag="stage")
                    nc.sync.dma_start(st[:], src_d[m * P:(m + 1) * P, :])
                    for k in range(n_k):
                        pt = tpsum.tile([P, P], f32, tag="tp")
                        nc.tensor.matmul(pt[:], st[:, k * P:(k + 1) * P], ident[:],
                                         start=True, stop=True)
                        off = (k * m_tiles + m) * P
                        nc.vector.tensor_copy(dst[:, off:off + P], pt[:])

                # m=0 staging first so the PE's first transposes aren't queued
                # behind 16 MiB of weight DMAs.
                transpose_rows(x_d, xT, ke, 0)
                transpose_rows(h_d, hT, kh, 0)
                for g in range(NG):
                    load_wu(g)

                for g in range(NG):
                    for m in range(m_tiles):
                        if g == 0 and m > 0:
                            # Interleave transposes with the first gate phase so the
                            # PE never sits idle waiting for all of xT/hT up front.
                            transpose_rows(x_d, xT, ke, m)
                            transpose_rows(h_d, hT, kh, m)
                        ps = gpsum.tile([P, hl], f32, tag="gp")
                        nc.tensor.matmul(ps[:], ones1[:, :],
                                         bias_row[:, g * hl:(g + 1) * hl],
                                         start=True, stop=False)
                        for k in range(ke):
                            nc.tensor.matmul(
                                ps[:], xT[:, (k * m_tiles + m) * P:(k * m_tiles + m + 1) * P],
                                w_sb[g][:, k, :], start=False, stop=False)
                        for k in range(kh):
                            nc.tensor.matmul(
                                ps[:], hT[:, (k * m_tiles + m) * P:(k * m_tiles + m + 1) * P],
                                u_sb[g][:, k, :], start=False, stop=(k == kh - 1))

                        ms = slice(m * hl, (m + 1) * hl)
                        if g == 0:
                            nc.scalar.activation(cbar[:, ms], ps[:], AFT.Tanh)
                        elif g == 1:
                            sig = work.tile([P, hl], f32, tag="sig")
                            nc.scalar.activation(sig[:], ps[:], AFT.Sigmoid)
                            nc.vector.tensor_mul(cbar[:, ms], sig[:], cbar[:, ms])
                        elif g == 2:
                            sig = work.tile([P, hl], f32, tag="sig")
                            nc.scalar.activation(sig[:], ps[:], AFT.Sigmoid)
                            if m == 0:
                                cp_all = persist.tile([P, m_tiles, hl], f32)
                                nc.sync.dma_start(
                                    cp_all[:], c_d.rearrange("(m p) c -> p m c", p=P))
                                emit_body.cp_all = cp_all
                            ct = work.tile([P, hl], f32, tag="outst")
                            nc.vector.tensor_mul(ct[:], sig[:], emit_body.cp_all[:, m, :])
                            nc.vector.tensor_add(ct[:], ct[:], cbar[:, ms])
                            nc.sync.dma_start(cout_d[m * P:(m + 1) * P, :], ct[:])
                            nc.scalar.activation(tnc[:, ms], ct[:], AFT.Tanh)
                        else:
                            sig = work.tile([P, hl], f32, tag="sig")
                            nc.scalar.activation(sig[:], ps[:], AFT.Sigmoid)
                            ht = work.tile([P, hl], f32, tag="outst")
                            nc.vector.tensor_mul(ht[:], sig[:], tnc[:, ms])
                            nc.sync.dma_start(hout_d[m * P:(m + 1) * P, :], ht[:])

            for _ in range(repeat):
                emit_body()
    return split_multiwaits(nc) if split else nc


_NC_CACHE = {}


def _get_nc(key=(BL, E, H, HL)):
    if key not in _NC_CACHE:
        _NC_CACHE[key] = build_nc(*key)
    return _NC_CACHE[key]


def make_in_maps(x, h_prev, c_prev, W, U, b):
    """W/U: [NG, E|H, H] stacked gate-major (cbar, i, f, o); b: [NG, H]."""
    in_maps = []
    for core in range(BB * BH):
        i, j = divmod(core, BH)
        rs = slice(i * BL, (i + 1) * BL)
        cs = slice(j * HL, (j + 1) * HL)
        in_maps.append({
            "x": np.ascontiguousarray(x[rs]),
            "h_prev": np.ascontiguousarray(h_prev[rs]),
            "c_prev": np.ascontiguousarray(c_prev[rs, cs]),
            "w": np.ascontiguousarray(W[:, :, cs]),
            "u": np.ascontiguousarray(U[:, :, cs]),
            "b": np.ascontiguousarray(b[:, cs]).reshape(1, NG * HL),
        })
    return in_maps


def kernel(**inputs):
    x = np.asarray(inputs["x"], np.float32)
    hm = np.asarray(inputs["hidden_memory_tm1"], np.float32)
    h_prev, c_prev = hm[0], hm[1]
    W = np.stack([np.asarray(inputs[k], np.float32)
                  for k in ("Wc", "Wi", "Wf", "Wog")])
    U = np.stack([np.asarray(inputs[k], np.float32)
                  for k in ("Uc", "Ui", "Uf", "Uog")])
    b = np.stack([np.asarray(inputs[k], np.float32)
                  for k in ("bc", "bi", "bf", "bog")])

    nc = _get_nc()
    res = run_bass_kernel_spmd(nc, make_in_maps(x, h_prev, c_prev, W, U, b),
                               list(range(BB * BH)))
    h = np.empty((B, H), np.float32)
    c = np.empty((B, H), np.float32)
    for core in range(BB * BH):
        i, j = divmod(core, BH)
        rs = slice(i * BL, (i + 1) * BL)
        cs = slice(j * HL, (j + 1) * HL)
        h[rs, cs] = res.results[core]["h_out"]
        c[rs, cs] = res.results[core]["c_out"]
    return np.stack([h, c])



# revision 3
# speedup vs baseline: 7.5692x; 7.5692x over previous
"""LSTM cell (4096x1024, H=1024) as a Bass/Tile kernel on 8 TRN2 NeuronCores.

Sharding: 2D grid — 4 batch-quarters x 2 H-halves. Core c = 2*i + j gets
batch rows [i*1024,(i+1)*1024) and gate-output columns [j*512,(j+1)*512).

Transposed bf16 formulation: the host pre-transposes activations so every
GEMM operand lands in SBUF in its natural matmul layout —
  gates_T[n, m] = sum_k W[k, n] * xT[k, m] + sum_k U[k, n] * hT[k, m]
with W/U k-tiles as the stationary operand (K on partitions, native [E, HL]
layout) and xT/hT as the moving operand. No on-chip transposes at all.
GEMM inputs are bf16 (1 PE cycle/row vs 4 for fp32); accumulation stays fp32
in PSUM. The per-gate bias rides the activation op as a per-partition bias AP
(gate-output dim is the partition dim here), so no K=1 seed matmuls.

Gate phases stream in weight-arrival order g = cbar, i, f, o; the gating
elementwise work attaches to each phase (i: cbar*=i; f: c = f*c_prev + cbar,
tanh(c); o: h = o*tanh(c)) so only the o-phase epilogue trails the last
matmul. h/c are produced transposed [HL, BL]; the host transposes back.
"""

import numpy as np
import ml_dtypes
from contextlib import ExitStack

import bass_rust
import concourse.bass as bass
import concourse.mybir as mybir
import concourse.tile as tile
from concourse.vector_clock import ScopedClock
from concourse.bass_utils import run_bass_kernel_spmd

f32 = mybir.dt.float32
bf16 = mybir.dt.bfloat16
AFT = mybir.ActivationFunctionType
P = 128

B, E, H = 4096, 1024, 1024
BB, BH = 4, 2              # batch quarters x H halves
BL = B // BB               # 1024 rows per core
HL = H // BH               # 512 gate cols per core
NG = 4                     # gate order: cbar, i, f, o
MBL = 512                  # moving-dim chunk per matmul (one fp32 PSUM bank)


class PatchedTC(tile.TileContext):
    # This neuronxcc's core_v3 CTRL (Drain/NoOp) struct carries no sync-wait
    # slots, so the Tile tail-drain's waits must ride on EVSEM instructions.
    def _drain_and_barrier(self, tick_clock, wait_clock):
        tmp = mybir.InstNoOp(name=f"I-{self.nc.next_id()}",
                             engine=mybir.EngineType.SP)
        wait_clock.add_sem_waits(tmp, ScopedClock({None: tick_clock.global_clock}))
        sync = tmp.sync_info
        if sync is not None:
            for w in sync.on_wait:
                sem = bass_rust.SemaphoreHandle(w.ant_name, w.id)
                self.nc.sync.wait_ge(sem, w.wait_value)
        self.nc.sync.drain()
        self.nc.all_engine_barrier()
        popped = self.nc._tile_sem_poison_stack.pop()
        assert popped is self._sem_poison
        self.nc.clear_and_free_semaphores(list(self.sems.allocated().values()))
        self.nc.all_engine_barrier()


_SPLIT_SEQ = [0]


def split_multiwaits(nc, default_max=1, opcode_max=None):
    """This walrus build accepts at most one sync wait per instruction (zero
    for CTRL-struct ops like Drain/NoOp). Move excess waits onto dedicated
    EventSemaphore instructions inserted just before, on the same engine —
    semantically identical on an in-order engine queue."""
    opcode_max = opcode_max or {"Drain": 0, "NoOp": 0}
    for fn in nc.m.functions:
        for blk in fn.blocks:
            cur = blk.instructions
            out, changed = [], False
            for ins in cur:
                si = ins.sync_info
                waits = list(si.on_wait) if si is not None and si.on_wait else []
                cap = opcode_max.get(ins.opcode, default_max)
                if len(waits) > cap:
                    keep = waits[len(waits) - cap:] if cap else []
                    spill = waits[:len(waits) - cap]
                    for w in spill:
                        _SPLIT_SEQ[0] += 1
                        ev = mybir.InstEventSemaphore(
                            name=f"I-evw{_SPLIT_SEQ[0]}", engine=ins.engine)
                        ev.sync_info = bass_rust.SyncInfo(
                            on_wait=[w], on_update=[])
                        out.append(ev)
                    ins.sync_info = bass_rust.SyncInfo(
                        on_wait=keep, on_update=list(si.on_update))
                    changed = True
                out.append(ins)
            if changed:
                blk.instructions = out
    return nc


def build_nc(bl=BL, e=E, h=H, hl=HL, split=True, repeat=1):
    ke, kh = e // P, h // P
    nn = hl // P               # gate-col tiles per gate (4)
    nm = bl // MBL             # moving halves per chain (2)
    nc = bass.Bass(target_bir_lowering=False)
    xt_d = nc.declare_dram_parameter("xt", [e, bl], bf16, isOutput=False)
    ht_d = nc.declare_dram_parameter("ht", [h, bl], bf16, isOutput=False)
    ct_d = nc.declare_dram_parameter("ct", [hl, bl], f32, isOutput=False)
    w_d = nc.declare_dram_parameter("w", [NG, e, hl], bf16, isOutput=False)
    u_d = nc.declare_dram_parameter("u", [NG, h, hl], bf16, isOutput=False)
    b_d = nc.declare_dram_parameter("b", [P, NG * nn], f32, isOutput=False)
    hout_d = nc.declare_dram_parameter("h_out", [hl, bl], f32, isOutput=True)
    cout_d = nc.declare_dram_parameter("c_out", [hl, bl], f32, isOutput=True)

    with PatchedTC(nc) as tc:
        with ExitStack() as ctx:
            persist = ctx.enter_context(tc.tile_pool(name="persist", bufs=1))
            wu = ctx.enter_context(tc.tile_pool(name="wu", bufs=1))
            gatep = ctx.enter_context(tc.tile_pool(name="gatep", bufs=1))
            work = ctx.enter_context(tc.tile_pool(name="work", bufs=2))
            outp = ctx.enter_context(tc.tile_pool(name="outp", bufs=2))
            gpsum = ctx.enter_context(
                tc.tile_pool(name="gpsum", bufs=4, space="PSUM"))

            def emit_body():
                xt = persist.tile([P, ke, bl], bf16)
                ht = persist.tile([P, kh, bl], bf16)
                ct = persist.tile([P, nn, bl], f32)
                bsb = persist.tile([P, NG * nn], f32)
                w_sb = [None] * NG
                u_sb = [None] * NG

                def load_w(g):
                    t = wu.tile([P, ke, hl], bf16, tag=f"w{g}")
                    nc.sync.dma_start(
                        t[:], w_d[g].rearrange("(k p) c -> p k c", p=P))
                    w_sb[g] = t

                def load_u(g):
                    t = wu.tile([P, kh, hl], bf16, tag=f"u{g}")
                    nc.sync.dma_start(
                        t[:], u_d[g].rearrange("(k p) c -> p k c", p=P))
                    u_sb[g] = t

                # DMA issue order ~ first-use order: the g=0 chains only need
                # xt + w0 (+ bias for the first activation); ht/u0 arrive while
                # the x-products stream.
                nc.sync.dma_start(xt[:], xt_d.rearrange("(k p) c -> p k c", p=P))
                load_w(0)
                nc.sync.dma_start(bsb[:], b_d[:, :])
                nc.sync.dma_start(ht[:], ht_d.rearrange("(k p) c -> p k c", p=P))
                load_u(0)
                for g in range(1, NG):
                    load_w(g)
                    load_u(g)
                nc.sync.dma_start(ct[:], ct_d.rearrange("(n p) c -> p n c", p=P))

                cb = [gatep.tile([P, bl], f32, tag=f"cb{n}", name=f"cb{n}")
                      for n in range(nn)]
                tnc = [gatep.tile([P, bl], f32, tag=f"tc{n}", name=f"tc{n}")
                       for n in range(nn)]

                for g in range(NG):
                    for n in range(nn):
                        ncol = slice(n * P, (n + 1) * P)
                        ps = gpsum.tile([P, bl], f32, tag="gp")
                        for m in range(nm):
                            mo = slice(m * MBL, (m + 1) * MBL)
                            for k in range(ke):
                                nc.tensor.matmul(
                                    ps[:, mo], w_sb[g][:, k, ncol],
                                    xt[:, k, mo], start=(k == 0), stop=False)
                            for k in range(kh):
                                nc.tensor.matmul(
                                    ps[:, mo], u_sb[g][:, k, ncol],
                                    ht[:, k, mo], start=False, stop=(k == kh - 1))
                        bias = bsb[:, g * nn + n:g * nn + n + 1]
                        if g == 0:
                            nc.scalar.activation(cb[n][:], ps[:], AFT.Tanh,
                                                 bias=bias)
                        elif g == 1:
                            it = work.tile([P, bl], f32, tag="it")
                            nc.scalar.activation(it[:], ps[:], AFT.Sigmoid,
                                                 bias=bias)
                            nc.vector.tensor_mul(cb[n][:], it[:], cb[n][:])
                        elif g == 2:
                            ft = work.tile([P, bl], f32, tag="ft")
                            nc.scalar.activation(ft[:], ps[:], AFT.Sigmoid,
                                                 bias=bias)
                            cblk = outp.tile([P, bl], f32, tag="co")
                            nc.vector.tensor_mul(cblk[:], ft[:], ct[:, n, :])
                            nc.vector.tensor_add(cblk[:], cblk[:], cb[n][:])
                            nc.sync.dma_start(cout_d[n * P:(n + 1) * P, :],
                                              cblk[:])
                            nc.scalar.activation(tnc[n][:], cblk[:], AFT.Tanh)
                        else:
                            ot = work.tile([P, bl], f32, tag="ot")
                            nc.scalar.activation(ot[:], ps[:], AFT.Sigmoid,
                                                 bias=bias)
                            hblk = outp.tile([P, bl], f32, tag="ho")
                            nc.vector.tensor_mul(hblk[:], ot[:], tnc[n][:])
                            nc.sync.dma_start(hout_d[n * P:(n + 1) * P, :],
                                              hblk[:])

            for _ in range(repeat):
                emit_body()
    return split_multiwaits(nc) if split else nc


_NC_CACHE = {}


def _get_nc(key=(BL, E, H, HL)):
    if key not in _NC_CACHE:
        _NC_CACHE[key] = build_nc(*key)
    return _NC_CACHE[key]


def make_in_maps(x, h_prev, c_prev, W, U, b):
    """W/U: [NG, E|H, H] stacked gate-major (cbar, i, f, o); b: [NG, H]."""
    bf = ml_dtypes.bfloat16
    nn = HL // P
    in_maps = []
    for core in range(BB * BH):
        i, j = divmod(core, BH)
        rs = slice(i * BL, (i + 1) * BL)
        cs = slice(j * HL, (j + 1) * HL)
        # bias as [128, NG*nn]: column t = g*nn + n holds the 128 bias values
        # for gate g, gate-col tile n — per-partition scalars for the ACT op.
        bcol = np.ascontiguousarray(b[:, cs]).reshape(NG * nn, P).T
        in_maps.append({
            "xt": np.ascontiguousarray(x[rs].T).astype(bf),
            "ht": np.ascontiguousarray(h_prev[rs].T).astype(bf),
            "ct": np.ascontiguousarray(c_prev[rs, cs].T),
            "w": np.ascontiguousarray(W[:, :, cs]).astype(bf),
            "u": np.ascontiguousarray(U[:, :, cs]).astype(bf),
            "b": np.ascontiguousarray(bcol),
        })
    return in_maps


def kernel(**inputs):
    x = np.asarray(inputs["x"], np.float32)
    hm = np.asarray(inputs["hidden_memory_tm1"], np.float32)
    h_prev, c_prev = hm[0], hm[1]
    W = np.stack([np.asarray(inputs[k], np.float32)
                  for k in ("Wc", "Wi", "Wf", "Wog")])
    U = np.stack([np.asarray(inputs[k], np.float32)
                  for k in ("Uc", "Ui", "Uf", "Uog")])
    b = np.stack([np.asarray(inputs[k], np.float32)
                  for k in ("bc", "bi", "bf", "bog")])

    nc = _get_nc()
    res = run_bass_kernel_spmd(nc, make_in_maps(x, h_prev, c_prev, W, U, b),
                               list(range(BB * BH)))
    h = np.empty((B, H), np.float32)
    c = np.empty((B, H), np.float32)
    for core in range(BB * BH):
        i, j = divmod(core, BH)
        rs = slice(i * BL, (i + 1) * BL)
        cs = slice(j * HL, (j + 1) * HL)
        h[rs, cs] = res.results[core]["h_out"].T
        c[rs, cs] = res.results[core]["c_out"].T
    return np.stack([h, c])
